# revision 14
# baseline (speedup 1.0000x reference)
"""Trainium2 Bass kernel for AlignmentModule (per-sample cross-attention).

Reference computation (per batch sample b):
    Q = W @ q + b            # (HID, HW)   1x1-conv channel matmul
    K = W @ p + b            # (HID, HW)
    S = Q^T K                # (HW, HW)
    A = softmax(S, axis=-1)
    aligned = V @ A^T        # (C, HW), V = p
    out = concat([q, aligned], channel axis)

Strategy: data-parallel over batch across 8 NeuronCores (2 samples/core).
All matmuls run on TensorE in float32r (full-rate fp32 storage); the AV
contraction runs in bf16 (A's exp weights + host-pretransposed V).
Softmax is computed on transposed scores S^T (k on partitions) so no
on-device transposes are needed: the column sum over k is a ones-matmul
on TensorE that also broadcasts the sum across all 128 partitions, and
the exp() has a constant -40 shift (scores are O(+-75), so exp never
overflows fp32/bf16 and softmax is shift-invariant).

The concat with raw query features is pure data movement and is done on
host during unsharding.
"""

import sys

if "/opt/trn_rl_repo" not in sys.path:
    sys.path.insert(0, "/opt/trn_rl_repo")

import ml_dtypes
import numpy as np

import concourse.bass as bass
import concourse.mybir as mybir
import concourse.tile as tile
from concourse import bacc
from concourse.bass_utils import run_bass_kernel_spmd

import os
F32_OUT = bool(int(os.environ.get("KERNEL_F32_OUT", "0")))
NO_WARMUP = bool(int(os.environ.get("KERNEL_NO_WARMUP", "0")))

B, C, HID, H, W_ = 16, 2048, 256, 32, 32
HW = H * W_            # 1024
NCORES = 8
BS = B // NCORES       # samples per core
P = 128
CT = C // P            # 16 channel tiles
OT = HID // P          # 2 hid blocks
KT = HW // P           # 8 key blocks
NH = 2                 # free-dim halves of HW
NF = HW // NH          # 512 (fp32 moving-operand max)

F32 = mybir.dt.float32
F32R = mybir.dt.float32r
BF16 = mybir.dt.bfloat16
EXP_SHIFT = -40.0

_NC_CACHE = None
LAST_RESULTS = None


def _ensure_ntff_hook():
    """Register the axon NTFF profile hook if the image's antenv lacks it.

    Profiling-only plumbing: run_bass_kernel_spmd(trace=True) under axon
    imports antenv.axon_hooks; some images ship antenv without that
    submodule even though the boot shim has the ctypes implementation.
    """
    import types

    try:
        from antenv.axon_hooks import get_axon_ntff_profile_hook  # noqa: F401
        return
    except ImportError:
        pass
    try:
        from trn_agent_boot.trn_boot import _ntff_profile_via_ctypes
    except ImportError:
        return
    hook = _ntff_profile_via_ctypes("/opt/axon/libaxon_pjrt.so")
    mod = types.ModuleType("antenv.axon_hooks")
    mod._hook = hook
    mod.get_axon_ntff_profile_hook = lambda: mod._hook
    mod.set_axon_ntff_profile_hook = lambda h: setattr(mod, "_hook", h)
    sys.modules["antenv.axon_hooks"] = mod
    import antenv

    antenv.axon_hooks = mod


def _build_nc():
    nc = bacc.Bacc(None, target_bir_lowering=False)

    q_d = nc.declare_dram_parameter("q", [BS, C, HW], F32R, isOutput=False)
    p_d = nc.declare_dram_parameter("p", [BS, C, HW], F32R, isOutput=False)
    pt_d = nc.declare_dram_parameter("pt", [BS, HW, C], BF16, isOutput=False)
    wt_d = nc.declare_dram_parameter("wt", [C, HID], F32R, isOutput=False)
    b_d = nc.declare_dram_parameter("b2", [P, OT], F32, isOutput=False)
    out_d = nc.declare_dram_parameter("out", [BS, C, HW], F32 if F32_OUT else BF16, isOutput=True)

    Ident = mybir.ActivationFunctionType.Identity
    Exp = mybir.ActivationFunctionType.Exp

    with tile.TileContext(nc) as tc:
        with (
            tc.tile_pool(name="const", bufs=1) as const_pool,
            tc.tile_pool(name="xstream", bufs=4) as x_pool,
            tc.tile_pool(name="vt", bufs=2) as vt_pool,
            tc.tile_pool(name="qf", bufs=1) as qf_pool,
            tc.tile_pool(name="kf", bufs=1) as kf_pool,
            tc.tile_pool(name="e", bufs=2) as e_pool,
            tc.tile_pool(name="rb", bufs=2) as rb_pool,
            tc.tile_pool(name="ostage", bufs=2) as o_pool,
            tc.tile_pool(name="acc_ps", bufs=1, space="PSUM") as acc_psum,
            tc.tile_pool(name="av_ps", bufs=4, space="PSUM") as av_psum,
        ):
            wt_r = wt_d.rearrange("(a p) o -> p a o", p=P)
            wt_s = const_pool.tile([P, CT, HID], F32R)
            for w4 in range(CT // 4):
                nc.sync.dma_start(
                    wt_s[:, 4 * w4:4 * (w4 + 1), :], wt_r[:, 4 * w4:4 * (w4 + 1), :]
                )
            b_s = const_pool.tile([P, OT], F32)
            nc.sync.dma_start(b_s[:], b_d[:])
            ones_s = const_pool.tile([P, P], BF16)
            nc.any.memset(ones_s[:], 1.0)
            shift_s = const_pool.tile([P, 1], F32)
            nc.any.memset(shift_s[:], EXP_SHIFT)

            # PE warm-up: ~9us of dummy matmuls so the HAM clock gate
            # opens (K=8/8) while the first projection stream is still
            # in flight, instead of ~25us into the kernel.
            if NO_WARMUP:
                wu_iters = 0
            else:
                wu_iters = 28
            wu_src = const_pool.tile([P, NF], BF16)
            nc.any.memset(wu_src[:], 0.0)
            wu_sink = const_pool.tile([P, 1], F32)
            if wu_iters:
                wu_ps = av_psum.tile([P, NF], F32, name="avp")
                for i in range(wu_iters):
                    nc.tensor.matmul(
                        wu_ps[:],
                        wu_src[:, :P],
                        wu_src[:],
                        start=(i == 0),
                        stop=(i == wu_iters - 1),
                    )
                nc.vector.tensor_copy(wu_sink[:], wu_ps[:, :1])

            for s in range(BS):
                # --- projections: Qf/Kf [o_p, j, hw] = W @ x + b ---
                # t-outer with 4 live PSUM accumulators (j x h); inputs
                # stream as [128, 2, 1024] pair-row tiles (4KB packets).
                qf = qf_pool.tile([P, OT, HW], F32R)
                kf = kf_pool.tile([P, OT, HW], F32R)
                for src, dst in ((q_d, qf), (p_d, kf)):
                    src_r = src[s].rearrange("(a p) f -> p a f", p=P)
                    pj = [
                        [
                            acc_psum.tile([P, NF], F32, name=f"A{2 * j + h}")
                            for h in range(NH)
                        ]
                        for j in range(OT)
                    ]
                    for u in range(CT // 2):
                        xt = x_pool.tile([P, 2, HW], F32R, name="xp")
                        nc.sync.dma_start(xt[:], src_r[:, 2 * u:2 * u + 2, :])
                        for du in range(2):
                            t = 2 * u + du
                            for j in range(OT):
                                for h in range(NH):
                                    nc.tensor.matmul(
                                        pj[j][h][:],
                                        wt_s[:, t, j * P:(j + 1) * P],
                                        xt[:, du, h * NF:(h + 1) * NF],
                                        start=(t == 0),
                                        stop=(t == CT - 1),
                                    )
                    for j in range(OT):
                        for h in range(NH):
                            nc.scalar.activation(
                                dst[:, j, h * NF:(h + 1) * NF],
                                pj[j][h][:],
                                Ident,
                                bias=b_s[:, j:j + 1],
                                scale=1.0,
                            )

                # --- scores^T + exp + softmax denominators ---
                # h-outer so each half's colsum + reciprocal overlaps the
                # next half's matmuls (the 4us DVE reciprocal otherwise
                # stalls the first AV evictions). Colsum MMs are staggered
                # one kb behind the S^T MMs to give the exp ACT slack.
                e = e_pool.tile([P, KT, HW], BF16)
                rb = rb_pool.tile([P, NH, NF], F32)
                for h in range(NH):
                    smp = acc_psum.tile([P, NF], F32, name=f"A{2 + h}")

                    def colsum(kb, h=h, smp=smp):
                        nc.tensor.matmul(
                            smp[:],
                            ones_s[:],
                            e[:, kb, h * NF:(h + 1) * NF],
                            start=(kb == 0),
                            stop=(kb == KT - 1),
                        )

                    for kb in range(KT):
                        stp = acc_psum.tile([P, NF], F32, name=f"A{kb % 2}")
                        for j in range(OT):
                            nc.tensor.matmul(
                                stp[:],
                                kf[:, j, kb * P:(kb + 1) * P],
                                qf[:, j, h * NF:(h + 1) * NF],
                                start=(j == 0),
                                stop=(j == OT - 1),
                            )
                        nc.scalar.activation(
                            e[:, kb, h * NF:(h + 1) * NF],
                            stp[:],
                            Exp,
                            bias=shift_s[:],
                            scale=1.0,
                        )
                        if kb >= 1:
                            colsum(kb - 1)
                    colsum(KT - 1)
                    nc.vector.reciprocal(rb[:, h, :], smp[:])

                    if h == 0:
                        # V^T tiles (host-transposed prompt, bf16):
                        # [hw_p, kb, c]. Emitted here so these DMAs queue
                        # after the projection streams (which gate S^T)
                        # but complete before AV consumes them.
                        vt = vt_pool.tile([P, KT, C], BF16)
                        pt_r = pt_d[s].rearrange("(a p) c -> p a c", p=P)
                        for v4 in range(KT // 2):
                            nc.sync.dma_start(
                                vt[:, 2 * v4:2 * v4 + 2, :],
                                pt_r[:, 2 * v4:2 * v4 + 2, :],
                            )

                # --- aligned[c_p, q] = (V E) * recip; paired 1MB out DMAs
                # on the ACT HWDGE ring ---
                out_r = out_d[s].rearrange("(a p) f -> p a f", p=P)
                for cp in range(CT // 2):
                    ot = o_pool.tile([P, 2, HW], F32 if F32_OUT else BF16, name="ot")
                    for dc in range(2):
                        cb = 2 * cp + dc
                        for h in range(NH):
                            avp = av_psum.tile([P, NF], F32, name="avp")
                            for kb in range(KT):
                                nc.tensor.matmul(
                                    avp[:],
                                    vt[:, kb, cb * P:(cb + 1) * P],
                                    e[:, kb, h * NF:(h + 1) * NF],
                                    start=(kb == 0),
                                    stop=(kb == KT - 1),
                                )
                            nc.vector.tensor_mul(
                                ot[:, dc, h * NF:(h + 1) * NF], avp[:], rb[:, h, :]
                            )
                    nc.scalar.dma_start(
                        out_r[:, 2 * cp:2 * cp + 2, :], ot[:]
                    )

    nc.compile()
    return nc


def _get_nc():
    global _NC_CACHE
    if _NC_CACHE is None:
        _NC_CACHE = _build_nc()
    return _NC_CACHE


def kernel(query_features, prompt_features, W, b, _profile=False):
    global LAST_RESULTS
    qv = np.ascontiguousarray(
        np.asarray(query_features, dtype=np.float32).reshape(B, C, HW)
    )
    pv = np.ascontiguousarray(
        np.asarray(prompt_features, dtype=np.float32).reshape(B, C, HW)
    )
    pt = np.ascontiguousarray(pv.transpose(0, 2, 1)).astype(ml_dtypes.bfloat16)
    wt = np.ascontiguousarray(np.asarray(W, dtype=np.float32).T)
    b2 = np.ascontiguousarray(np.asarray(b, dtype=np.float32).reshape(OT, P).T)

    if _profile:
        _ensure_ntff_hook()
    nc = _get_nc()
    in_maps = []
    for i in range(NCORES):
        sl = slice(i * BS, (i + 1) * BS)
        in_maps.append(
            {"q": qv[sl], "p": pv[sl], "pt": pt[sl], "wt": wt, "b2": b2}
        )
    res = run_bass_kernel_spmd(
        nc, in_maps, core_ids=list(range(NCORES)), trace=_profile
    )
    LAST_RESULTS = res
    aligned = np.concatenate(
        [np.asarray(r["out"], dtype=np.float32) for r in res.results], axis=0
    )
    aligned = aligned.reshape(B, C, H, W_)
    full = np.concatenate(
        [np.asarray(query_features, dtype=np.float32).reshape(B, C, H, W_), aligned],
        axis=1,
    )
    return full


# revision 15
# speedup vs baseline: 1.1074x; 1.1074x over previous
"""Trainium2 Bass kernel for AlignmentModule (per-sample cross-attention).

Reference computation (per batch sample b):
    Q = W @ q + b            # (HID, HW)   1x1-conv channel matmul
    K = W @ p + b            # (HID, HW)
    S = Q^T K                # (HW, HW)
    A = softmax(S, axis=-1)
    aligned = V @ A^T        # (C, HW), V = p
    out = concat([q, aligned], channel axis)

Strategy: data-parallel over batch across 8 NeuronCores (2 samples/core).
All matmuls run on TensorE in float32r (full-rate fp32 storage); the AV
contraction runs in bf16 (A's exp weights + host-pretransposed V).
Softmax is computed on transposed scores S^T (k on partitions) so no
on-device transposes are needed: the column sum over k is a ones-matmul
on TensorE that also broadcasts the sum across all 128 partitions, and
the exp() has a constant -40 shift (scores are O(+-75), so exp never
overflows fp32/bf16 and softmax is shift-invariant).

The concat with raw query features is pure data movement and is done on
host during unsharding.
"""

import sys

if "/opt/trn_rl_repo" not in sys.path:
    sys.path.insert(0, "/opt/trn_rl_repo")

import ml_dtypes
import numpy as np

import concourse.bass as bass
import concourse.mybir as mybir
import concourse.tile as tile
from concourse import bacc
from concourse.bass_utils import run_bass_kernel_spmd

import os
F32_OUT = bool(int(os.environ.get("KERNEL_F32_OUT", "1")))
NO_WARMUP = bool(int(os.environ.get("KERNEL_NO_WARMUP", "0")))

B, C, HID, H, W_ = 16, 2048, 256, 32, 32
HW = H * W_            # 1024
NCORES = 8
BS = B // NCORES       # samples per core
P = 128
CT = C // P            # 16 channel tiles
OT = HID // P          # 2 hid blocks
KT = HW // P           # 8 key blocks
NH = 2                 # free-dim halves of HW
NF = HW // NH          # 512 (fp32 moving-operand max)

F32 = mybir.dt.float32
F32R = mybir.dt.float32r
BF16 = mybir.dt.bfloat16
EXP_SHIFT = -40.0

_NC_CACHE = None
LAST_RESULTS = None


def _ensure_ntff_hook():
    """Register the axon NTFF profile hook if the image's antenv lacks it.

    Profiling-only plumbing: run_bass_kernel_spmd(trace=True) under axon
    imports antenv.axon_hooks; some images ship antenv without that
    submodule even though the boot shim has the ctypes implementation.
    """
    import types

    try:
        from antenv.axon_hooks import get_axon_ntff_profile_hook  # noqa: F401
        return
    except ImportError:
        pass
    try:
        from trn_agent_boot.trn_boot import _ntff_profile_via_ctypes
    except ImportError:
        return
    hook = _ntff_profile_via_ctypes("/opt/axon/libaxon_pjrt.so")
    mod = types.ModuleType("antenv.axon_hooks")
    mod._hook = hook
    mod.get_axon_ntff_profile_hook = lambda: mod._hook
    mod.set_axon_ntff_profile_hook = lambda h: setattr(mod, "_hook", h)
    sys.modules["antenv.axon_hooks"] = mod
    import antenv

    antenv.axon_hooks = mod


def _build_nc():
    nc = bacc.Bacc(None, target_bir_lowering=False)

    q_d = nc.declare_dram_parameter("q", [BS, C, HW], F32R, isOutput=False)
    p_d = nc.declare_dram_parameter("p", [BS, C, HW], F32R, isOutput=False)
    pt_d = nc.declare_dram_parameter("pt", [BS, HW, C], BF16, isOutput=False)
    wt_d = nc.declare_dram_parameter("wt", [C, HID], F32R, isOutput=False)
    b_d = nc.declare_dram_parameter("b2", [P, OT], F32, isOutput=False)
    out_d = nc.declare_dram_parameter("out", [BS, C, HW], F32 if F32_OUT else BF16, isOutput=True)

    Ident = mybir.ActivationFunctionType.Identity
    Exp = mybir.ActivationFunctionType.Exp

    with tile.TileContext(nc) as tc:
        with (
            tc.tile_pool(name="const", bufs=1) as const_pool,
            tc.tile_pool(name="xstream", bufs=4) as x_pool,
            tc.tile_pool(name="vt", bufs=2) as vt_pool,
            tc.tile_pool(name="qf", bufs=1) as qf_pool,
            tc.tile_pool(name="kf", bufs=1) as kf_pool,
            tc.tile_pool(name="e", bufs=2) as e_pool,
            tc.tile_pool(name="rb", bufs=2) as rb_pool,
            tc.tile_pool(name="ostage", bufs=2) as o_pool,
            tc.tile_pool(name="acc_ps", bufs=1, space="PSUM") as acc_psum,
            tc.tile_pool(name="av_ps", bufs=4, space="PSUM") as av_psum,
        ):
            wt_r = wt_d.rearrange("(a p) o -> p a o", p=P)
            wt_s = const_pool.tile([P, CT, HID], F32R)
            for w4 in range(CT // 4):
                nc.sync.dma_start(
                    wt_s[:, 4 * w4:4 * (w4 + 1), :], wt_r[:, 4 * w4:4 * (w4 + 1), :]
                )
            b_s = const_pool.tile([P, OT], F32)
            nc.sync.dma_start(b_s[:], b_d[:])
            ones_s = const_pool.tile([P, P], BF16)
            nc.any.memset(ones_s[:], 1.0)
            shift_s = const_pool.tile([P, 1], F32)
            nc.any.memset(shift_s[:], EXP_SHIFT)

            # PE warm-up: ~9us of dummy matmuls so the HAM clock gate
            # opens (K=8/8) while the first projection stream is still
            # in flight, instead of ~25us into the kernel.
            if NO_WARMUP:
                wu_iters = 0
            else:
                wu_iters = 28
            wu_src = const_pool.tile([P, NF], BF16)
            nc.any.memset(wu_src[:], 0.0)
            wu_sink = const_pool.tile([P, 1], F32)
            if wu_iters:
                wu_ps = av_psum.tile([P, NF], F32, name="avp")
                for i in range(wu_iters):
                    nc.tensor.matmul(
                        wu_ps[:],
                        wu_src[:, :P],
                        wu_src[:],
                        start=(i == 0),
                        stop=(i == wu_iters - 1),
                    )
                nc.vector.tensor_copy(wu_sink[:], wu_ps[:, :1])

            for s in range(BS):
                # --- projections: Qf/Kf [o_p, j, hw] = W @ x + b ---
                # t-outer with 4 live PSUM accumulators (j x h); inputs
                # stream as [128, 2, 1024] pair-row tiles (4KB packets).
                qf = qf_pool.tile([P, OT, HW], F32R)
                kf = kf_pool.tile([P, OT, HW], F32R)
                for src, dst in ((q_d, qf), (p_d, kf)):
                    src_r = src[s].rearrange("(a p) f -> p a f", p=P)
                    pj = [
                        [
                            acc_psum.tile([P, NF], F32, name=f"A{2 * j + h}")
                            for h in range(NH)
                        ]
                        for j in range(OT)
                    ]
                    for u in range(CT // 2):
                        xt = x_pool.tile([P, 2, HW], F32R, name="xp")
                        nc.sync.dma_start(xt[:], src_r[:, 2 * u:2 * u + 2, :])
                        for du in range(2):
                            t = 2 * u + du
                            for j in range(OT):
                                for h in range(NH):
                                    nc.tensor.matmul(
                                        pj[j][h][:],
                                        wt_s[:, t, j * P:(j + 1) * P],
                                        xt[:, du, h * NF:(h + 1) * NF],
                                        start=(t == 0),
                                        stop=(t == CT - 1),
                                    )
                    for j in range(OT):
                        for h in range(NH):
                            nc.scalar.activation(
                                dst[:, j, h * NF:(h + 1) * NF],
                                pj[j][h][:],
                                Ident,
                                bias=b_s[:, j:j + 1],
                                scale=1.0,
                            )

                # --- scores^T + exp + softmax denominators ---
                # h-outer so each half's colsum + reciprocal overlaps the
                # next half's matmuls (the 4us DVE reciprocal otherwise
                # stalls the first AV evictions). Colsum MMs are staggered
                # one kb behind the S^T MMs to give the exp ACT slack.
                e = e_pool.tile([P, KT, HW], BF16)
                rb = rb_pool.tile([P, NH, NF], F32)
                for h in range(NH):
                    smp = acc_psum.tile([P, NF], F32, name=f"A{2 + h}")

                    def colsum(kb, h=h, smp=smp):
                        nc.tensor.matmul(
                            smp[:],
                            ones_s[:],
                            e[:, kb, h * NF:(h + 1) * NF],
                            start=(kb == 0),
                            stop=(kb == KT - 1),
                        )

                    for kb in range(KT):
                        stp = acc_psum.tile([P, NF], F32, name=f"A{kb % 2}")
                        for j in range(OT):
                            nc.tensor.matmul(
                                stp[:],
                                kf[:, j, kb * P:(kb + 1) * P],
                                qf[:, j, h * NF:(h + 1) * NF],
                                start=(j == 0),
                                stop=(j == OT - 1),
                            )
                        nc.scalar.activation(
                            e[:, kb, h * NF:(h + 1) * NF],
                            stp[:],
                            Exp,
                            bias=shift_s[:],
                            scale=1.0,
                        )
                        if kb >= 1:
                            colsum(kb - 1)
                    colsum(KT - 1)
                    nc.vector.reciprocal(rb[:, h, :], smp[:])

                    if h == 0:
                        # V^T tiles (host-transposed prompt, bf16):
                        # [hw_p, kb, c]. Emitted here so these DMAs queue
                        # after the projection streams (which gate S^T)
                        # but complete before AV consumes them.
                        vt = vt_pool.tile([P, KT, C], BF16)
                        pt_r = pt_d[s].rearrange("(a p) c -> p a c", p=P)
                        for v4 in range(KT // 2):
                            nc.sync.dma_start(
                                vt[:, 2 * v4:2 * v4 + 2, :],
                                pt_r[:, 2 * v4:2 * v4 + 2, :],
                            )

                # --- aligned[c_p, q] = (V E) * recip; paired 1MB out DMAs
                # on the ACT HWDGE ring ---
                out_r = out_d[s].rearrange("(a p) f -> p a f", p=P)
                for cp in range(CT // 2):
                    ot = o_pool.tile([P, 2, HW], F32 if F32_OUT else BF16, name="ot")
                    for dc in range(2):
                        cb = 2 * cp + dc
                        for h in range(NH):
                            avp = av_psum.tile([P, NF], F32, name="avp")
                            for kb in range(KT):
                                nc.tensor.matmul(
                                    avp[:],
                                    vt[:, kb, cb * P:(cb + 1) * P],
                                    e[:, kb, h * NF:(h + 1) * NF],
                                    start=(kb == 0),
                                    stop=(kb == KT - 1),
                                )
                            nc.vector.tensor_mul(
                                ot[:, dc, h * NF:(h + 1) * NF], avp[:], rb[:, h, :]
                            )
                    nc.scalar.dma_start(
                        out_r[:, 2 * cp:2 * cp + 2, :], ot[:]
                    )

    nc.compile()
    return nc


def _get_nc():
    global _NC_CACHE
    if _NC_CACHE is None:
        _NC_CACHE = _build_nc()
    return _NC_CACHE


def kernel(query_features, prompt_features, W, b, _profile=False):
    global LAST_RESULTS
    qv = np.ascontiguousarray(
        np.asarray(query_features, dtype=np.float32).reshape(B, C, HW)
    )
    pv = np.ascontiguousarray(
        np.asarray(prompt_features, dtype=np.float32).reshape(B, C, HW)
    )
    pt = np.ascontiguousarray(pv.transpose(0, 2, 1)).astype(ml_dtypes.bfloat16)
    wt = np.ascontiguousarray(np.asarray(W, dtype=np.float32).T)
    b2 = np.ascontiguousarray(np.asarray(b, dtype=np.float32).reshape(OT, P).T)

    if _profile:
        _ensure_ntff_hook()
    nc = _get_nc()
    in_maps = []
    for i in range(NCORES):
        sl = slice(i * BS, (i + 1) * BS)
        in_maps.append(
            {"q": qv[sl], "p": pv[sl], "pt": pt[sl], "wt": wt, "b2": b2}
        )
    res = run_bass_kernel_spmd(
        nc, in_maps, core_ids=list(range(NCORES)), trace=_profile
    )
    LAST_RESULTS = res
    aligned = np.concatenate(
        [np.asarray(r["out"], dtype=np.float32) for r in res.results], axis=0
    )
    aligned = aligned.reshape(B, C, H, W_)
    full = np.concatenate(
        [np.asarray(query_features, dtype=np.float32).reshape(B, C, H, W_), aligned],
        axis=1,
    )
    return full


# revision 16
# speedup vs baseline: 1.1986x; 1.0823x over previous
"""Trainium2 Bass kernel for AlignmentModule (per-sample cross-attention).

Reference computation (per batch sample b):
    Q = W @ q + b            # (HID, HW)   1x1-conv channel matmul
    K = W @ p + b            # (HID, HW)
    S = Q^T K                # (HW, HW)
    A = softmax(S, axis=-1)
    aligned = V @ A^T        # (C, HW), V = p
    out = concat([q, aligned], channel axis)

Strategy: data-parallel over batch across 8 NeuronCores (2 samples/core).
All matmuls run on TensorE in float32r (full-rate fp32 storage); the AV
contraction runs in bf16 (A's exp weights + host-pretransposed V).
Softmax is computed on transposed scores S^T (k on partitions) so no
on-device transposes are needed: the column sum over k is a ones-matmul
on TensorE that also broadcasts the sum across all 128 partitions, and
the exp() has a constant -40 shift (scores are O(+-75), so exp never
overflows fp32/bf16 and softmax is shift-invariant).

The concat with raw query features is pure data movement and is done on
host during unsharding.
"""

import sys

if "/opt/trn_rl_repo" not in sys.path:
    sys.path.insert(0, "/opt/trn_rl_repo")

import ml_dtypes
import numpy as np

import concourse.bass as bass
import concourse.mybir as mybir
import concourse.tile as tile
from concourse import bacc
from concourse.bass_utils import run_bass_kernel_spmd

import os
F32_OUT = bool(int(os.environ.get("KERNEL_F32_OUT", "1")))
NO_WARMUP = bool(int(os.environ.get("KERNEL_NO_WARMUP", "0")))

B, C, HID, H, W_ = 16, 2048, 256, 32, 32
HW = H * W_            # 1024
NCORES = 8
BS = B // NCORES       # samples per core
P = 128
CT = C // P            # 16 channel tiles
OT = HID // P          # 2 hid blocks
KT = HW // P           # 8 key blocks
NH = 2                 # free-dim halves of HW
NF = HW // NH          # 512 (fp32 moving-operand max)

F32 = mybir.dt.float32
F32R = mybir.dt.float32r
BF16 = mybir.dt.bfloat16
EXP_SHIFT = -40.0

_NC_CACHE = None
LAST_RESULTS = None


def _ensure_ntff_hook():
    """Register the axon NTFF profile hook if the image's antenv lacks it.

    Profiling-only plumbing: run_bass_kernel_spmd(trace=True) under axon
    imports antenv.axon_hooks; some images ship antenv without that
    submodule even though the boot shim has the ctypes implementation.
    """
    import types

    try:
        from antenv.axon_hooks import get_axon_ntff_profile_hook  # noqa: F401
        return
    except ImportError:
        pass
    try:
        from trn_agent_boot.trn_boot import _ntff_profile_via_ctypes
    except ImportError:
        return
    hook = _ntff_profile_via_ctypes("/opt/axon/libaxon_pjrt.so")
    mod = types.ModuleType("antenv.axon_hooks")
    mod._hook = hook
    mod.get_axon_ntff_profile_hook = lambda: mod._hook
    mod.set_axon_ntff_profile_hook = lambda h: setattr(mod, "_hook", h)
    sys.modules["antenv.axon_hooks"] = mod
    import antenv

    antenv.axon_hooks = mod


def _build_nc():
    nc = bacc.Bacc(None, target_bir_lowering=False)

    q_d = nc.declare_dram_parameter("q", [BS, C, HW], F32R, isOutput=False)
    p_d = nc.declare_dram_parameter("p", [BS, C, HW], F32R, isOutput=False)
    pt_d = nc.declare_dram_parameter("pt", [BS, HW, C], BF16, isOutput=False)
    wt_d = nc.declare_dram_parameter("wt", [C, HID], F32R, isOutput=False)
    b_d = nc.declare_dram_parameter("b2", [P, OT], F32, isOutput=False)
    out_d = nc.declare_dram_parameter("out", [BS, C, HW], F32 if F32_OUT else BF16, isOutput=True)

    Ident = mybir.ActivationFunctionType.Identity
    Exp = mybir.ActivationFunctionType.Exp

    with tile.TileContext(nc) as tc:
        with (
            tc.tile_pool(name="const", bufs=1) as const_pool,
            tc.tile_pool(name="xstream", bufs=6) as x_pool,
            tc.tile_pool(name="vt", bufs=1) as vt_pool,
            tc.tile_pool(name="qf", bufs=1) as qf_pool,
            tc.tile_pool(name="kf", bufs=1) as kf_pool,
            tc.tile_pool(name="e", bufs=2) as e_pool,
            tc.tile_pool(name="rb", bufs=2) as rb_pool,
            tc.tile_pool(name="ostage", bufs=3) as o_pool,
            tc.tile_pool(name="acc_ps", bufs=1, space="PSUM") as acc_psum,
            tc.tile_pool(name="av_ps", bufs=4, space="PSUM") as av_psum,
        ):
            wt_r = wt_d.rearrange("(a p) o -> p a o", p=P)
            wt_s = const_pool.tile([P, CT, HID], F32R)
            for w4 in range(CT // 4):
                nc.sync.dma_start(
                    wt_s[:, 4 * w4:4 * (w4 + 1), :], wt_r[:, 4 * w4:4 * (w4 + 1), :]
                )
            b_s = const_pool.tile([P, OT], F32)
            nc.sync.dma_start(b_s[:], b_d[:])
            ones_s = const_pool.tile([P, P], BF16)
            nc.any.memset(ones_s[:], 1.0)
            shift_s = const_pool.tile([P, 1], F32)
            nc.any.memset(shift_s[:], EXP_SHIFT)

            # PE warm-up: ~9us of dummy matmuls so the HAM clock gate
            # opens (K=8/8) while the first projection stream is still
            # in flight, instead of ~25us into the kernel.
            if NO_WARMUP:
                wu_iters = 0
            else:
                wu_iters = 28
            wu_src = const_pool.tile([P, NF], BF16)
            nc.any.memset(wu_src[:], 0.0)
            wu_sink = const_pool.tile([P, 1], F32)
            if wu_iters:
                wu_ps = av_psum.tile([P, NF], F32, name="avp")
                for i in range(wu_iters):
                    nc.tensor.matmul(
                        wu_ps[:],
                        wu_src[:, :P],
                        wu_src[:],
                        start=(i == 0),
                        stop=(i == wu_iters - 1),
                    )
                nc.vector.tensor_copy(wu_sink[:], wu_ps[:, :1])

            for s in range(BS):
                # --- projections: Qf/Kf [o_p, j, hw] = W @ x + b ---
                # t-outer with 4 live PSUM accumulators (j x h); inputs
                # stream as [128, 2, 1024] pair-row tiles (4KB packets).
                qf = qf_pool.tile([P, OT, HW], F32R)
                kf = kf_pool.tile([P, OT, HW], F32R)
                for src, dst in ((q_d, qf), (p_d, kf)):
                    src_r = src[s].rearrange("(a p) f -> p a f", p=P)
                    pj = [
                        [
                            acc_psum.tile([P, NF], F32, name=f"A{2 * j + h}")
                            for h in range(NH)
                        ]
                        for j in range(OT)
                    ]
                    for u in range(CT // 2):
                        xt = x_pool.tile([P, 2, HW], F32R, name="xp")
                        nc.sync.dma_start(xt[:], src_r[:, 2 * u:2 * u + 2, :])
                        for du in range(2):
                            t = 2 * u + du
                            for j in range(OT):
                                for h in range(NH):
                                    nc.tensor.matmul(
                                        pj[j][h][:],
                                        wt_s[:, t, j * P:(j + 1) * P],
                                        xt[:, du, h * NF:(h + 1) * NF],
                                        start=(t == 0),
                                        stop=(t == CT - 1),
                                    )
                    for j in range(OT):
                        for h in range(NH):
                            nc.scalar.activation(
                                dst[:, j, h * NF:(h + 1) * NF],
                                pj[j][h][:],
                                Ident,
                                bias=b_s[:, j:j + 1],
                                scale=1.0,
                            )

                # --- scores^T + exp + softmax denominators ---
                # h-outer so each half's colsum + reciprocal overlaps the
                # next half's matmuls (the 4us DVE reciprocal otherwise
                # stalls the first AV evictions). Colsum MMs are staggered
                # one kb behind the S^T MMs to give the exp ACT slack.
                e = e_pool.tile([P, KT, HW], BF16)
                rb = rb_pool.tile([P, NH, NF], F32)
                for h in range(NH):
                    smp = acc_psum.tile([P, NF], F32, name=f"A{2 + h}")

                    def colsum(kb, h=h, smp=smp):
                        nc.tensor.matmul(
                            smp[:],
                            ones_s[:],
                            e[:, kb, h * NF:(h + 1) * NF],
                            start=(kb == 0),
                            stop=(kb == KT - 1),
                        )

                    for kb in range(KT):
                        stp = acc_psum.tile([P, NF], F32, name=f"A{kb % 2}")
                        for j in range(OT):
                            nc.tensor.matmul(
                                stp[:],
                                kf[:, j, kb * P:(kb + 1) * P],
                                qf[:, j, h * NF:(h + 1) * NF],
                                start=(j == 0),
                                stop=(j == OT - 1),
                            )
                        nc.scalar.activation(
                            e[:, kb, h * NF:(h + 1) * NF],
                            stp[:],
                            Exp,
                            bias=shift_s[:],
                            scale=1.0,
                        )
                        if kb >= 1:
                            colsum(kb - 1)
                    colsum(KT - 1)
                    nc.vector.reciprocal(rb[:, h, :], smp[:])

                    if h == 0:
                        # V^T tiles (host-transposed prompt, bf16):
                        # [hw_p, kb, c]. Emitted here so these DMAs queue
                        # after the projection streams (which gate S^T)
                        # but complete before AV consumes them.
                        vt = vt_pool.tile([P, KT, C], BF16)
                        pt_r = pt_d[s].rearrange("(a p) c -> p a c", p=P)
                        for v4 in range(KT // 2):
                            nc.sync.dma_start(
                                vt[:, 2 * v4:2 * v4 + 2, :],
                                pt_r[:, 2 * v4:2 * v4 + 2, :],
                            )

                # --- aligned[c_p, q] = (V E) * recip; paired 1MB out DMAs
                # on the ACT HWDGE ring ---
                out_r = out_d[s].rearrange("(a p) f -> p a f", p=P)
                for cp in range(CT // 2):
                    ot = o_pool.tile([P, 2, HW], F32 if F32_OUT else BF16, name="ot")
                    for dc in range(2):
                        cb = 2 * cp + dc
                        for h in range(NH):
                            avp = av_psum.tile([P, NF], F32, name="avp")
                            for kb in range(KT):
                                nc.tensor.matmul(
                                    avp[:],
                                    vt[:, kb, cb * P:(cb + 1) * P],
                                    e[:, kb, h * NF:(h + 1) * NF],
                                    start=(kb == 0),
                                    stop=(kb == KT - 1),
                                )
                            nc.vector.tensor_mul(
                                ot[:, dc, h * NF:(h + 1) * NF], avp[:], rb[:, h, :]
                            )
                    nc.scalar.dma_start(
                        out_r[:, 2 * cp:2 * cp + 2, :], ot[:]
                    )

    nc.compile()
    return nc


def _get_nc():
    global _NC_CACHE
    if _NC_CACHE is None:
        _NC_CACHE = _build_nc()
    return _NC_CACHE


def kernel(query_features, prompt_features, W, b, _profile=False):
    global LAST_RESULTS
    qv = np.ascontiguousarray(
        np.asarray(query_features, dtype=np.float32).reshape(B, C, HW)
    )
    pv = np.ascontiguousarray(
        np.asarray(prompt_features, dtype=np.float32).reshape(B, C, HW)
    )
    pt = np.ascontiguousarray(pv.transpose(0, 2, 1)).astype(ml_dtypes.bfloat16)
    wt = np.ascontiguousarray(np.asarray(W, dtype=np.float32).T)
    b2 = np.ascontiguousarray(np.asarray(b, dtype=np.float32).reshape(OT, P).T)

    if _profile:
        _ensure_ntff_hook()
    nc = _get_nc()
    in_maps = []
    for i in range(NCORES):
        sl = slice(i * BS, (i + 1) * BS)
        in_maps.append(
            {"q": qv[sl], "p": pv[sl], "pt": pt[sl], "wt": wt, "b2": b2}
        )
    res = run_bass_kernel_spmd(
        nc, in_maps, core_ids=list(range(NCORES)), trace=_profile
    )
    LAST_RESULTS = res
    aligned = np.concatenate(
        [np.asarray(r["out"], dtype=np.float32) for r in res.results], axis=0
    )
    aligned = aligned.reshape(B, C, H, W_)
    full = np.concatenate(
        [np.asarray(query_features, dtype=np.float32).reshape(B, C, H, W_), aligned],
        axis=1,
    )
    return full
